# revision 3
# baseline (speedup 1.0000x reference)
"""Trainium2 Bass kernel for DLRANet (4-layer low-rank MLP + log_softmax).

Strategy:
- Data-parallel over 8 NeuronCores: each core computes 1024 rows of the
  8192-row batch; the small low-rank factors K_i/Vt_i are replicated.
- Low-rank fused: never materializes W_i = K_i @ Vt_i. Per hidden layer,
  h = z @ K (contraction) and z' = relu(h @ Vt) (expansion) are computed
  chunk-by-chunk over the 4096-wide hidden dim, so the [B,4096]
  activations never hit DRAM and only small chunks live in PSUM/SBUF.
- Activations are kept feature-major ("transposed": [feature, batch]) so
  every matmul consumes K_i / Vt_i in their natural layouts; x.T is
  prepared host-side during sharding. The final layer flips back to
  batch-major (activation chunk becomes the stationary operand), which
  makes the fused log_softmax a row-wise op.
- All matmuls run in fp16 (1 PE row/cycle).
- Elementwise throughput is the second-binding constraint (GpSimd cannot
  read PSUM, so only Scalar+Vector can run the 4096-wide relus). In the
  first two transitions relus are PAIRED (one [128,1024] op covers two
  w-chunks, halving per-op overhead); assignment rotates between Scalar
  and Vector with phase-specific weights so neither engine gates PE.
- PSUM is re-laid-out per phase with two scoped tile pools:
  P1 (L0+t0+t1): 2x hacc bank + 3x two-bank paired-z tiles.
  P2 (t2+final): 1x hacc + 3x single-bank z tiles + 2x two-bank logits.
- Input DMA triggers cost ~0.7us of engine queue time each, so they are
  spread across three queues in need-order: Scalar HWDGE carries k0/vt0,
  GpSimd SWDGE carries k1/vt1, Sync carries x (graded chunk sizes so
  layer 0 starts asap) then k2/vt2/k3/vt3 and the output tiles.
"""

import numpy as np

_B, _DIN, _WID, _DOUT, _R = 8192, 1024, 4096, 1000, 128
_NC = 8
_BL = _B // _NC  # rows per core
_NB = 512  # batch sub-chunk (moving-operand free dim)
_NBC = _BL // _NB  # sub-chunks per core (2)
_DCH = _DIN // 128  # d-chunks in layer 0 (8)
_WCH = _WID // 128  # w-chunks per hidden layer (32)

_cache = {}


def _chunk_major(a, p=128):
    """[C*p, F] -> [p, C*F]: partition-major layout for one contiguous DMA."""
    c = a.shape[0] // p
    return np.ascontiguousarray(
        a.reshape(c, p, a.shape[1]).transpose(1, 0, 2).reshape(p, c * a.shape[1])
    )


def build(reps=1):
    import os
    import concourse.bacc as bacc
    import concourse.mybir as mybir
    import concourse.tile as tile

    t2lag = int(os.environ.get("KB_T2LAG", "2"))

    F16 = mybir.dt.float16
    F32 = mybir.dt.float32
    AF = mybir.ActivationFunctionType

    nc = bacc.Bacc(trn_type="TRN2", target_bir_lowering=False, debug=False)

    xT_d = nc.dram_tensor("xT", [128, _DCH * _BL], F16, kind="ExternalInput").ap()
    k_d = [
        nc.dram_tensor(
            f"k{i}",
            [128, (_DCH if i == 0 else _WCH) * _R],
            F16,
            kind="ExternalInput",
        ).ap()
        for i in range(4)
    ]
    vt_d = [
        nc.dram_tensor(
            f"vt{i}", [128, _WID if i < 3 else _DOUT], F16, kind="ExternalInput"
        ).ap()
        for i in range(4)
    ]
    out_d = nc.dram_tensor("out", [_BL, _DOUT], F32, kind="ExternalOutput").ap()

    with tile.TileContext(nc) as tc:
        with tc.tile_pool(name="wp", bufs=1) as wp, tc.tile_pool(
            name="hp", bufs=1
        ) as hp, tc.tile_pool(name="zp", bufs=1) as zp, tc.tile_pool(
            name="fp", bufs=1
        ) as fp:

            def body():
                # ---- input DMAs across three queues, need-order ----
                # Scalar HWDGE: k0 halves + vt0 quarters (earliest weights;
                # Scalar's first relu isn't until ~14us in)
                k0h = []
                for h in range(2):
                    kh = wp.tile([128, _DCH // 2, _R], F16, tag=f"k0h{h}", name=f"k0h{h}")
                    nc.scalar.dma_start(
                        kh[:],
                        k_d[0][
                            :, h * (_DCH // 2) * _R : (h + 1) * (_DCH // 2) * _R
                        ].rearrange("p (c r) -> p c r", c=_DCH // 2),
                    )
                    k0h.append(kh)
                # Sync: x in graded chunk sizes (units of NB=512 columns)
                xTh = {}
                for lo, hi in [(0, 1), (1, 4), (4, 8), (8, 16)]:
                    xt = wp.tile(
                        [128, (hi - lo) * _NB], F16, tag=f"xT{lo}", name=f"xT{lo}"
                    )
                    nc.sync.dma_start(xt[:], xT_d[:, lo * _NB : hi * _NB])
                    for u in range(lo, hi):
                        c, bc = u // _NBC, u % _NBC
                        xTh[(c, bc)] = xt[:, (u - lo) * _NB : (u - lo + 1) * _NB]

                NQ = 4  # quarters per 4096-wide tensor
                vt_q = [[None] * NQ for _ in range(3)]
                kn_q = [[None] * NQ for _ in range(3)]

                def load_w(i, q, eng, vt_only=False, k_only=False):
                    if not k_only:
                        v = wp.tile(
                            [128, _WID // NQ], F16, tag=f"vt{i}q{q}", name=f"vt{i}q{q}"
                        )
                        eng.dma_start(
                            v[:], vt_d[i][:, q * (_WID // NQ) : (q + 1) * (_WID // NQ)]
                        )
                        vt_q[i][q] = v
                    if not vt_only:
                        k = wp.tile(
                            [128, _WCH // NQ, _R],
                            F16,
                            tag=f"k{i+1}q{q}",
                            name=f"k{i+1}q{q}",
                        )
                        eng.dma_start(
                            k[:],
                            k_d[i + 1][
                                :, q * (_WID // NQ) : (q + 1) * (_WID // NQ)
                            ].rearrange("p (c r) -> p c r", c=_WCH // NQ),
                        )
                        kn_q[i][q] = k

                for q in range(NQ):  # vt0 on Scalar
                    load_w(0, q, nc.scalar, vt_only=True)
                for q in range(NQ):  # k1 on GpSimd (SWDGE)
                    load_w(0, q, nc.gpsimd, k_only=True)
                for q in range(NQ):  # vt1 on GpSimd
                    load_w(1, q, nc.gpsimd, vt_only=True)
                for q in range(NQ):  # k2 + vt2 interleaved on Sync (after x)
                    load_w(1, q, nc.sync, k_only=True)
                    load_w(2, q, nc.sync, vt_only=True)
                for q in range(NQ):  # k3 on Sync
                    load_w(2, q, nc.sync, k_only=True)
                vt3_s = wp.tile([128, _DOUT], F16, tag="vt3s", name="vt3s")
                nc.sync.dma_start(vt3_s[:], vt_d[3][:])
                WQ = _WCH // NQ  # w-chunks per quarter (8)

                # weighted scalar/vector rotation for relus
                rl = {"s": 0.0, "v": 0.0}

                def emit_relu(zt, pz, w_scalar=1.0, w_vector=1.0):
                    if rl["s"] / w_scalar <= rl["v"] / w_vector:
                        rl["s"] += 1.0
                        nc.scalar.activation(zt[:], pz[:], AF.Relu)
                    else:
                        rl["v"] += 1.0
                        nc.vector.tensor_scalar_max(zt[:], pz[:], 0.0)

                def emit_hcopy(ht, hacc):
                    # split PSUM->SBUF h copy in halves on two engines so the
                    # next layer's first z-matmul unblocks earlier
                    nc.scalar.copy(ht[:, 0 : _NB // 2], hacc[:, 0 : _NB // 2])
                    nc.vector.tensor_copy(
                        ht[:, _NB // 2 : _NB], hacc[:, _NB // 2 : _NB]
                    )

                # ================= P1: layer 0 + transitions 0,1 =============
                with tc.tile_pool(name="ps1", bufs=1, space="PSUM") as ps1:
                    hacc = [
                        ps1.tile([128, _NB], F32, tag="hacc", bufs=2, name=f"hacc0_{bc}")
                        for bc in range(_NBC)
                    ]
                    for c in range(_DCH):
                        for bc in range(_NBC):
                            nc.tensor.matmul(
                                hacc[bc][:],
                                k0h[c // (_DCH // 2)][:, c % (_DCH // 2), :],
                                xTh[(c, bc)][:],
                                start=(c == 0),
                                stop=(c == _DCH - 1),
                            )
                    h_cur = []
                    for bc in range(_NBC):
                        ht = hp.tile([128, _NB], F16, tag="h", bufs=6, name=f"h0_{bc}")
                        emit_hcopy(ht, hacc[bc])
                        h_cur.append(ht)

                    # transitions 0,1: w-chunk PAIRS, both batch sub-chunks
                    # interleaved; one [128,1024] relu covers a pair; h-matmuls
                    # run one pair behind.
                    NP = _WCH // 2  # pairs (16)
                    for t in range(2):
                        hacc = [
                            ps1.tile(
                                [128, _NB], F32, tag="hacc", bufs=2,
                                name=f"hacc{t+1}_{bc}",
                            )
                            for bc in range(_NBC)
                        ]
                        zs_live = {}
                        for p in range(NP + 1):
                            if p < NP:
                                for bc in range(_NBC):
                                    pz = ps1.tile(
                                        [128, 2 * _NB], F32, tag="pzp", bufs=3,
                                        name=f"pz{t}_{p}_{bc}",
                                    )
                                    for half in range(2):
                                        wc = 2 * p + half
                                        nc.tensor.matmul(
                                            pz[:, half * _NB : (half + 1) * _NB],
                                            vt_q[t][wc // WQ][
                                                :, (wc % WQ) * 128 : (wc % WQ + 1) * 128
                                            ],
                                            h_cur[bc][:],
                                            start=True,
                                            stop=True,
                                        )
                                    zt = zp.tile(
                                        [128, 2 * _NB], F16, tag="zs", bufs=4,
                                        name=f"zs{t}_{p}_{bc}",
                                    )
                                    emit_relu(zt, pz)
                                    zs_live[(p, bc)] = zt
                            if p >= 1:
                                for bc in range(_NBC):
                                    zt = zs_live.pop((p - 1, bc))
                                    for half in range(2):
                                        wc = 2 * (p - 1) + half
                                        nc.tensor.matmul(
                                            hacc[bc][:],
                                            kn_q[t][wc // WQ][:, wc % WQ, :],
                                            zt[:, half * _NB : (half + 1) * _NB],
                                            start=(wc == 0),
                                            stop=(wc == _WCH - 1),
                                        )
                        h_nxt = []
                        for bc in range(_NBC):
                            ht = hp.tile(
                                [128, _NB], F16, tag="h", bufs=6, name=f"h{t+1}_{bc}"
                            )
                            emit_hcopy(ht, hacc[bc])
                            h_nxt.append(ht)
                        h_cur = h_nxt

                # ============== P2: transition 2 + final layer ===============
                # final layer + log_softmax for one 128-row batch chunk
                def emit_final_chunk(g, h3_tile, j, fin_pool):
                    lhsT = h3_tile[:, j * 128 : (j + 1) * 128]
                    lgp = fin_pool.tile(
                        [128, 2 * _NB], F32, tag="fin", bufs=2, name=f"lgp{g}"
                    )
                    nc.tensor.matmul(
                        lgp[:, 0:_NB], lhsT, vt3_s[:, 0:_NB], start=True, stop=True
                    )
                    nc.tensor.matmul(
                        lgp[:, _NB:_DOUT], lhsT, vt3_s[:, _NB:_DOUT],
                        start=True, stop=True,
                    )
                    lg = lgp[:, 0:_DOUT]
                    o_s = fp.tile([128, _DOUT], F32, tag="os", bufs=3, name=f"os{g}")
                    e_s = fp.tile([128, _DOUT], F32, tag="e", bufs=2, name=f"e{g}")
                    ssum = fp.tile([128, 1], F32, tag="ss", bufs=2, name=f"ss{g}")
                    nc.scalar.activation(e_s[:], lg[:], AF.Exp, accum_out=ssum[:])
                    lns = fp.tile([128, 1], F32, tag="lns", bufs=2, name=f"lns{g}")
                    nc.scalar.activation(lns[:], ssum[:], AF.Ln)
                    # subtract in halves (both on Vector; frees lgp sooner)
                    nc.vector.tensor_scalar_sub(
                        o_s[:, 0:_NB], lg[:, 0:_NB], lns[:]
                    )
                    nc.vector.tensor_scalar_sub(
                        o_s[:, _NB:_DOUT], lg[:, _NB:_DOUT], lns[:]
                    )
                    nc.sync.dma_start(out_d[g * 128 : (g + 1) * 128, :], o_s[:])

                with tc.tile_pool(name="ps2", bufs=1, space="PSUM") as ps2:
                    for bc in range(_NBC):
                        hacc3 = ps2.tile(
                            [128, _NB], F32, tag="hacc1", bufs=1, name=f"hacc3_{bc}"
                        )
                        zs_live = {}
                        for wc in range(_WCH + t2lag):
                            if wc < _WCH:
                                pz = ps2.tile(
                                    [128, _NB], F32, tag="pz1", bufs=3,
                                    name=f"pzt2_{bc}_{wc}",
                                )
                                nc.tensor.matmul(
                                    pz[:],
                                    vt_q[2][wc // WQ][
                                        :, (wc % WQ) * 128 : (wc % WQ + 1) * 128
                                    ],
                                    h_cur[bc][:],
                                    start=True,
                                    stop=True,
                                )
                                zt = zp.tile(
                                    [128, _NB], F16, tag="zs1", bufs=6,
                                    name=f"zt2_{bc}_{wc}",
                                )
                                # during bc1's w-loop Scalar also runs bc0's
                                # softmax exps; weight relus toward Vector
                                if bc == 0:
                                    emit_relu(zt, pz)
                                else:
                                    emit_relu(zt, pz, w_scalar=0.75, w_vector=1.25)
                                zs_live[wc] = zt
                            if wc >= t2lag:
                                nc.tensor.matmul(
                                    hacc3[:],
                                    kn_q[2][(wc - t2lag) // WQ][:, (wc - t2lag) % WQ, :],
                                    zs_live.pop(wc - t2lag)[:],
                                    start=(wc == t2lag),
                                    stop=(wc == _WCH + t2lag - 1),
                                )
                        h3 = hp.tile([128, _NB], F16, tag="h", bufs=6, name=f"h3_{bc}")
                        emit_hcopy(h3, hacc3)
                        for j in range(_NB // 128):
                            emit_final_chunk(bc * (_NB // 128) + j, h3, j, ps2)

            if reps == 1:
                body()
            else:
                with tc.For_i(0, reps):
                    body()

    # All activation funcs used here (Relu/Copy/Identity/Exp/Ln) coexist in
    # act-func-set "natural_log_exp_and_others". Left alone, the table-load
    # pass picks the first set containing each func (exp->set0, ln->set5),
    # thrashing ~1.3us table loads between them. Restrict every other set's
    # advertised funcs so all activations resolve to that one set -> a single
    # table load for the whole kernel.
    import concourse.bacc as bacc_mod
    from concourse.hw_specs import get_activation_tables as _real_tables

    def _pinned_tables(arch):
        tabs = _real_tables(arch)
        pinned = "natural_log_exp_and_others"
        if pinned in tabs:
            ours = tabs[pinned]
            tabs = {
                name: (funcs if name == pinned else (funcs - ours))
                for name, funcs in tabs.items()
            }
        return tabs

    bacc_mod.get_activation_tables = _pinned_tables
    try:
        nc.compile()
    finally:
        bacc_mod.get_activation_tables = _real_tables
    return nc


def _prep_inputs(x, K0, Vt0, K1, Vt1, K2, Vt2, K3, Vt3):
    """Host-side sharding + layout prep: cast to fp16, chunk-major weights,
    per-core transposed x shards."""
    cast = lambda a: np.asarray(a, np.float32).astype(np.float16)
    ks = [_chunk_major(cast(np.asarray(k, np.float32))) for k in (K0, K1, K2, K3)]
    vts = [cast(np.ascontiguousarray(v, np.float32)) for v in (Vt0, Vt1, Vt2, Vt3)]
    xr = cast(np.asarray(x, np.float32))
    in_maps = []
    for c in range(_NC):
        xT = _chunk_major(np.ascontiguousarray(xr[c * _BL : (c + 1) * _BL].T))
        m = {"xT": xT}
        for i in range(4):
            m[f"k{i}"] = ks[i]
            m[f"vt{i}"] = vts[i]
        in_maps.append(m)
    return in_maps


def kernel(x, K0, Vt0, K1, Vt1, K2, Vt2, K3, Vt3):
    from concourse import bass_utils

    if "nc" not in _cache:
        _cache["nc"] = build(reps=1)
    nc = _cache["nc"]
    in_maps = _prep_inputs(x, K0, Vt0, K1, Vt1, K2, Vt2, K3, Vt3)
    res = bass_utils.run_bass_kernel_spmd(nc, in_maps, core_ids=list(range(_NC)))
    return np.concatenate([r["out"] for r in res.results], axis=0)
